# revision 1
# baseline (speedup 1.0000x reference)
"""FlowNet correlation kernel for Trainium2 (8 NeuronCores, batch-parallel).

Problem: out[b, d, y, x] = (1/C) * sum_c i1[b,c,y,x] * pad(i2)[b,c,y+dy,x+dx]
  B=8, C=256, H=48, W=64, pad=20, displacements dy,dx in {-20..20 step 2}
  (21x21 = 441), output [8, 441, 48, 64] fp32.

Strategy (per core, one batch element):
  Displacement stride 2 => the problem splits into 4 independent polyphase
  subproblems (y-parity sy, x-parity sx), each a dense +-10 correlation on a
  24x32 quarter image. For each subproblem and each block of 4 sub-rows
  (M = 4*32 = 128 output pixels), compute the all-pairs band via fp32
  matmuls: stationary = i1 block [C, 128], streaming = the padded-i2 window
  (24 sub-rows x 52 sub-cols = 1248 cols, split 468/468/312 to respect the
  512-fp32 PSUM bank limit), accumulating over the two 128-channel k-tiles.
  Scale by 1/C during the PSUM->SBUF copy, then extract the 441 per-pixel
  correlation values with diagonal-access-pattern DMAs (flat SBUF addressing
  couples partition and byte strides) writing directly to HBM in
  [y, x, d] layout (1764-byte contiguous runs). Host transposes to [d, y, x].
"""

import numpy as np

C = 256
H, W = 48, 64
ND = 21          # displacements per axis
D = ND * ND      # 441
SUB_H, SUB_W = H // 2, W // 2      # 24, 32
HP, WP = H + 40, W + 40            # padded full-res 88, 104
BAND_W = 52                        # padded sub-cols
BAND_ROWS = 24                     # window sub-rows per block
BAND_N = BAND_ROWS * BAND_W        # 1248
ROW_SPLITS = [(0, 9), (9, 18), (18, 24)]   # window-row ranges per PSUM bank
N_BLOCKS = SUB_H // 4              # 6

_CACHE = {}


def _build():
    import concourse.bacc as bacc
    import concourse.mybir as mybir
    from concourse.bass_types import AP, SBTensorHandle
    from concourse.tile import TileContext

    f32 = mybir.dt.float32

    def alias_sbuf(nc, name, shape, dtype, offset, base_partition):
        # SBUF tensor view at a fixed byte offset and nonzero base partition.
        # Mirrors alloc_sbuf_tensor_at but rebases the partition origin so
        # diagonal gather APs keep their flat offset inside one partition row
        # (walrus rejects partition-crossing offsets on irregular APs).
        uname = nc._get_name(name, add_next_id=True)
        nc._tensor(uname, list(shape), dtype, type="SB")
        import functools, operator
        per_part = functools.reduce(operator.mul, shape[1:]) * mybir.dt.size(dtype)
        h = SBTensorHandle(
            uname,
            list(shape),
            dtype,
            base_partition=base_partition,
            manual_sbuf_range=(offset, offset + per_part),
            manual_base_name=name,
        )
        mloc = nc.lookup_mloc(h)
        mloc.allocated = True
        mloc.addr = offset
        mloc.base = base_partition
        return h
    nc = bacc.Bacc("TRN2", target_bir_lowering=False, debug=False)
    i1_t = nc.dram_tensor("i1", [C, H, W], f32, kind="ExternalInput")
    i2_t = nc.dram_tensor("i2", [C, H, W], f32, kind="ExternalInput")
    od_t = nc.dram_tensor("od", [H, W, D], f32, kind="ExternalOutput")

    NBUF = 3
    band_full = []
    band_alias = []
    for i in range(NBUF):
        h = nc.alloc_sbuf_tensor(f"bandf{i}", [128, BAND_N], f32)
        addr = nc.lookup_mloc(h).addr
        band_full.append(h)
        band_alias.append(
            [
                alias_sbuf(nc, f"band{i}ry{ry}", [32, BAND_N], f32, addr, 32 * ry)
                for ry in range(4)
            ]
        )

    from bass_rust import add_dep_helper

    last_gathers = [[] for _ in range(NBUF)]

    with TileContext(nc) as tc:
        with (
            tc.tile_pool(name="inp", bufs=1) as inp_pool,
            tc.tile_pool(name="ps", bufs=2, space="PSUM") as ps_pool,
        ):
            i1_sb = [
                inp_pool.tile([128, H * W], f32, name=f"i1k{k}", tag=f"i1k{k}") for k in range(2)
            ]
            i2_sb = [
                inp_pool.tile([128, HP * WP], f32, name=f"i2k{k}", tag=f"i2k{k}") for k in range(2)
            ]
            i1s_sb = [
                [
                    inp_pool.tile(
                        [128, SUB_H * SUB_W], f32, name=f"i1s{k}{s}", tag=f"i1s{k}{s}"
                    )
                    for s in range(4)
                ]
                for k in range(2)
            ]
            i1v = [t[:].rearrange("c (h w) -> c h w", h=H) for t in i1_sb]
            i2v = [t[:].rearrange("c (h w) -> c h w", h=HP) for t in i2_sb]

            for k in range(2):
                cs = slice(128 * k, 128 * (k + 1))
                nc.sync.dma_start(out=i1_sb[k][:], in_=i1_t.ap()[cs])
                v = i2v[k]
                # zero the pad ring (gpsimd; disjoint from the interior DMA)
                nc.gpsimd.memset(v[:, 0:20, :], 0.0)
                nc.gpsimd.memset(v[:, 68:HP, :], 0.0)
                nc.gpsimd.memset(v[:, 20:68, 0:20], 0.0)
                nc.gpsimd.memset(v[:, 20:68, 84:WP], 0.0)
                nc.sync.dma_start(out=v[:, 20:68, 20:84], in_=i2_t.ap()[cs])
                # de-interleave i1 into the 4 polyphase sub-images (gpsimd):
                # stationary matmul operands need a single-stride free dim
                for s in range(4):
                    sy, sx = s >> 1, s & 1
                    nc.gpsimd.tensor_copy(
                        i1s_sb[k][s][:].rearrange(
                            "c (py px) -> c py px", py=SUB_H
                        ),
                        i1v[k][:, sy : sy + 2 * SUB_H - 1 : 2, sx::2],
                    )

            inv_c = 1.0 / C
            for s in range(4):
                sy, sx = s >> 1, s & 1
                for yb in range(N_BLOCKS):
                    Y = 4 * yb
                    ps = ps_pool.tile([128, 1536], f32, name="ps")
                    for j, (r0, r1) in enumerate(ROW_SPLITS):
                        n = (r1 - r0) * BAND_W
                        for k in range(2):
                            lhs = i1s_sb[k][s][:, 32 * Y : 32 * Y + 128]
                            rh = i2v[k][
                                :,
                                2 * (Y + r0) + sy : 2 * (Y + r1 - 1) + sy + 1 : 2,
                                sx::2,
                            ]
                            nc.tensor.matmul(
                                ps[:, 512 * j : 512 * j + n],
                                lhsT=lhs,
                                rhs=rh,
                                start=(k == 0),
                                stop=(k == 1),
                            )
                    bi = (s * N_BLOCKS + yb) % NBUF
                    band = band_full[bi].ap()
                    copies = [
                        nc.vector.tensor_scalar_mul(
                            band[:, 0:468], ps[:, 0:468], inv_c
                        ),
                        nc.vector.tensor_scalar_mul(
                            band[:, 468:936], ps[:, 512:980], inv_c
                        ),
                        nc.scalar.mul(band[:, 936:1248], ps[:, 1024:1336], inv_c),
                    ]
                    # band buffers live outside the tile pools (the gather
                    # aliases rebase partitions, which Tile can't track), so
                    # RAW (gather-after-copy) and WAR (copy-after-gather on
                    # buffer reuse) edges are added explicitly.
                    for c in copies:
                        for g in last_gathers[bi]:
                            add_dep_helper(c.ins, g.ins, reason="band WAR")
                    gathers = []
                    for ry in range(4):
                        rd = AP(
                            band_alias[bi][ry],
                            ry * BAND_W,
                            [[BAND_N + 1, 32], [BAND_W, ND], [1, ND]],
                        )
                        wr = AP(
                            od_t.ap().tensor,
                            (2 * (Y + ry) + sy) * (W * D) + sx * D,
                            [[2 * D, 32], [ND, ND], [1, ND]],
                        )
                        g = nc.sync.dma_start(out=wr, in_=rd)
                        for c in copies:
                            add_dep_helper(g.ins, c.ins, reason="band RAW")
                        gathers.append(g)
                    last_gathers[bi] = gathers

    nc.compile()
    return nc


def _get_program():
    if "nc" not in _CACHE:
        _CACHE["nc"] = _build()
    return _CACHE["nc"]


def kernel(input1: np.ndarray, input2: np.ndarray) -> np.ndarray:
    from concourse import bass_utils

    nc = _get_program()
    input1 = np.ascontiguousarray(input1, dtype=np.float32)
    input2 = np.ascontiguousarray(input2, dtype=np.float32)
    B = input1.shape[0]
    in_maps = [{"i1": input1[b], "i2": input2[b]} for b in range(B)]
    res = bass_utils.run_bass_kernel_spmd(nc, in_maps, core_ids=list(range(B)))
    out = np.stack([r["od"] for r in res.results])  # [B, H, W, D]
    return np.ascontiguousarray(out.transpose(0, 3, 1, 2))  # [B, D, H, W]



# revision 7
# speedup vs baseline: 1.5426x; 1.5426x over previous
"""FlowNet correlation kernel for Trainium2 (8 NeuronCores, batch-parallel).

Problem: out[b, d, y, x] = (1/C) * sum_c i1[b,c,y,x] * pad(i2)[b,c,y+dy,x+dx]
  B=8, C=256, H=48, W=64, pad=20, displacements dy,dx in {-20..20 step 2}
  (21x21 = 441), output [8, 441, 48, 64] fp32.

Strategy (per core, one batch element):
  Displacement stride 2 => 4 polyphase subproblems (y-parity sy, x-parity
  sx), each a dense +-10 correlation on a 24x32 quarter image. Inputs are
  cast to bf16 during the load DMA (tolerance is 2e-2 rms; bf16 keeps it
  ~6e-3). For each subproblem and block of 4 sub-rows (M = 128 pixels),
  bf16 matmuls compute the all-pairs band restricted to live (in-image)
  window positions only -- no pad ring in SBUF; structural zeros come from
  six geometry-keyed band buffers whose dead regions are zeroed once.

  Extraction to [y, x, d] HBM layout is 3-stage to avoid the 84-byte
  descriptor storm that a direct diagonal gather DMA costs (descriptor
  generation at ~3ns/descriptor serializes on the issuing engine):
    A. SBUF->SBUF DMA px-diagonal shift (flat addressing couples partition
       and byte strides): sel1[p, j] = band[p, px + j], long contiguous
       runs, live row span only (dead spans stay zero from a one-time
       memset of the six geometry-keyed sel1 buffers).
    B. compute-engine copies (partition-uniform per 32-lane py-group):
       out_sb[p, oy*21+ox] = sel1[p, (py+oy)*52 + ox], bf16.
    C. one linear DMA per block: out_sb [128, 441] bf16 -> HBM bf16 with
       882-byte runs (128 descriptors). The band was already rounded to
       bf16, so a bf16 HBM output loses nothing; the host widens to fp32
       (exact) and transposes [H, W, D] -> [D, H, W].
"""

import numpy as np

C = 256
H, W = 48, 64
ND = 21          # displacements per axis
D = ND * ND      # 441
SUB_H, SUB_W = H // 2, W // 2      # 24, 32
BAND_W = 52                        # window cols per band row
BAND_ROWS = 24                     # window rows per block
BAND_N = BAND_ROWS * BAND_W        # 1248
SEL_N = (BAND_ROWS - 1) * BAND_W + ND  # 1217: max j read from sel1, +1
N_BLOCKS = SUB_H // 4              # 6
YS = [0, 4, 8, 12, 16, 20]
# live window-row range [wr0, wr1) per y-block (rows with in-image data)
LIVE = [(max(0, 10 - Y), min(24, 34 - Y)) for Y in YS]

_CACHE = {}


def _build():
    import concourse.bacc as bacc
    import concourse.mybir as mybir
    from concourse.bass_types import AP, SBTensorHandle
    from concourse.tile import TileContext
    from bass_rust import add_dep_helper

    f32 = mybir.dt.float32
    bf16 = mybir.dt.bfloat16

    def alias_sbuf(nc, name, shape, dtype, offset, base_partition):
        # SBUF tensor view at a fixed byte offset and nonzero base partition.
        # Rebases the partition origin so diagonal gather APs keep their flat
        # offset inside one partition row (walrus rejects partition-crossing
        # offsets on irregular APs).
        uname = nc._get_name(name, add_next_id=True)
        nc._tensor(uname, list(shape), dtype, type="SB")
        import functools, operator
        per_part = functools.reduce(operator.mul, shape[1:]) * mybir.dt.size(dtype)
        h = SBTensorHandle(
            uname,
            list(shape),
            dtype,
            base_partition=base_partition,
            manual_sbuf_range=(offset, offset + per_part),
            manual_base_name=name,
        )
        mloc = nc.lookup_mloc(h)
        mloc.allocated = True
        mloc.addr = offset
        mloc.base = base_partition
        return h

    nc = bacc.Bacc("TRN2", target_bir_lowering=False, debug=False)
    i1_t = nc.dram_tensor("i1", [C, H, W], f32, kind="ExternalInput")
    i2_t = nc.dram_tensor("i2", [C, H, W], f32, kind="ExternalInput")
    od_t = nc.dram_tensor("od", [H, W, D], bf16, kind="ExternalOutput")

    # geometry-keyed band + sel1 buffers (raw tensors; deps tracked manually)
    band = []
    band_alias = []
    sel1 = []
    for g in range(N_BLOCKS):
        h = nc.alloc_sbuf_tensor(f"band{g}", [128, BAND_N], bf16)
        addr = nc.lookup_mloc(h).addr
        band.append(h)
        band_alias.append(
            [
                alias_sbuf(nc, f"band{g}ry{ry}", [32, BAND_N], bf16, addr, 32 * ry)
                for ry in range(4)
            ]
        )
        sel1.append(nc.alloc_sbuf_tensor(f"sel1_{g}", [128, SEL_N], bf16))

    # last readers of band[g] (stage-A DMAs) / sel1[g] (stage-B copies);
    # seeded with the zeroing memsets so first writers wait for them.
    band_last = [[] for _ in range(N_BLOCKS)]
    sel1_last = [[] for _ in range(N_BLOCKS)]

    inv_c = 1.0 / C

    with TileContext(nc) as tc:
        with (
            tc.tile_pool(name="inp", bufs=1) as inp_pool,
            tc.tile_pool(name="out", bufs=3) as out_pool,
            tc.tile_pool(name="ps", bufs=3, space="PSUM") as ps_pool,
        ):
            i1b = [
                inp_pool.tile([128, H * W], bf16, name=f"i1b{k}", tag=f"i1b{k}")
                for k in range(2)
            ]
            i2b = [
                inp_pool.tile([128, H * W], bf16, name=f"i2b{k}", tag=f"i2b{k}")
                for k in range(2)
            ]
            i1s = [
                [
                    inp_pool.tile(
                        [128, SUB_H * SUB_W], bf16, name=f"i1s{k}{s}", tag=f"i1s{k}{s}"
                    )
                    for s in range(4)
                ]
                for k in range(2)
            ]
            i1v = [t[:].rearrange("c (h w) -> c h w", h=H) for t in i1b]
            i2v = [t[:].rearrange("c (h w) -> c h w", h=H) for t in i2b]

            # input loads: fp32 HBM -> bf16 SBUF, cast by SWDGE (gpsimd)
            for k in range(2):
                cs = slice(128 * k, 128 * (k + 1))
                nc.gpsimd.dma_start(out=i1b[k][:], in_=i1_t.ap()[cs])
                nc.gpsimd.dma_start(out=i2b[k][:], in_=i2_t.ap()[cs])
                # de-interleave i1 into the 4 polyphase sub-images
                # (stationary matmul operands need a single-stride free dim)
                for s in range(4):
                    sy, sx = s >> 1, s & 1
                    eng = nc.vector if (s & 1) else nc.gpsimd
                    eng.tensor_copy(
                        i1s[k][s][:].rearrange("c (py px) -> c py px", py=SUB_H),
                        i1v[k][:, sy : sy + 2 * SUB_H - 1 : 2, sx::2],
                    )

            # zero band + sel1 buffers once (dead regions must read as 0)
            for g in range(N_BLOCKS):
                eng = nc.vector if (g & 1) else nc.gpsimd
                mb = eng.memset(band[g].ap(), 0.0)
                ms = eng.memset(sel1[g].ap(), 0.0)
                band_last[g] = [mb]
                sel1_last[g] = [ms]

            for s in range(4):
                sy, sx = s >> 1, s & 1
                for g, Y in enumerate(YS):
                    wr0, wr1 = LIVE[g]
                    nr = wr1 - wr0
                    # psum chunks of <= 16 window rows (512 fp32 bank limit)
                    chunks = [(a, min(a + 16, nr)) for a in range(0, nr, 16)]
                    ps = ps_pool.tile([128, 1024], f32, name="ps")
                    for k in range(2):
                        lhs = i1s[k][s][:, 32 * Y : 32 * Y + 128]
                        for j, (a, b) in enumerate(chunks):
                            r = Y + wr0 + a - 10  # first interior sub-row
                            rhs = i2v[k][
                                :,
                                2 * r + sy : 2 * (r + b - a - 1) + sy + 1 : 2,
                                sx::2,
                            ]
                            nc.tensor.matmul(
                                ps[:, 512 * j : 512 * j + (b - a) * 32],
                                lhsT=lhs,
                                rhs=rhs,
                                start=(k == 0),
                                stop=(k == 1),
                            )
                    # psum -> band (scale 1/C, cast to bf16); live region only
                    copies = []
                    for j, (a, b) in enumerate(chunks):
                        eng = nc.vector if (j == 0) else nc.scalar
                        dst = AP(
                            band[g],
                            (wr0 + a) * BAND_W + 10,
                            [[BAND_N, 128], [BAND_W, b - a], [1, 32]],
                        )
                        src = ps[:, 512 * j : 512 * j + (b - a) * 32].rearrange(
                            "c (r w) -> c r w", r=b - a
                        )
                        if eng is nc.vector:
                            cp = eng.tensor_scalar_mul(dst, src, inv_c)
                        else:
                            cp = eng.mul(dst, src, inv_c)
                        for rd in band_last[g]:
                            add_dep_helper(cp.ins, rd.ins, reason="band WAR")
                        copies.append(cp)
                    # stage A: px-diagonal shift band -> sel1 (live row span)
                    span = (nr - 1) * BAND_W + ND
                    gathers = []
                    for ry in range(4):
                        rd = AP(
                            band_alias[g][ry],
                            wr0 * BAND_W,
                            [[BAND_N + 1, 32], [1, span]],
                        )
                        wr = AP(
                            sel1[g],
                            (32 * ry) * SEL_N + wr0 * BAND_W,
                            [[SEL_N, 32], [1, span]],
                        )
                        dma = nc.sync.dma_start(out=wr, in_=rd)
                        for cp in copies:
                            add_dep_helper(dma.ins, cp.ins, reason="band RAW")
                        for rb in sel1_last[g]:
                            add_dep_helper(dma.ins, rb.ins, reason="sel1 WAR")
                        gathers.append(dma)
                    band_last[g] = gathers
                    # stage B: py-group-uniform window extract, sel1 -> out_sb
                    osb = out_pool.tile([128, D], bf16, name="osb")
                    readers = []
                    for ry in range(4):
                        src = AP(
                            sel1[g],
                            (32 * ry) * SEL_N + ry * BAND_W,
                            [[SEL_N, 32], [BAND_W, ND], [1, ND]],
                        )
                        dst = osb[32 * ry : 32 * ry + 32, :].rearrange(
                            "p (a b) -> p a b", a=ND
                        )
                        if ry == 3:
                            cp = nc.scalar.copy(dst, src)
                        else:
                            eng = nc.gpsimd if ry == 1 else nc.vector
                            cp = eng.tensor_copy(dst, src)
                        for dma in gathers:
                            add_dep_helper(cp.ins, dma.ins, reason="sel1 RAW")
                        readers.append(cp)
                    sel1_last[g] = readers
                    # stage C: linear store to HBM [y, x, d] bf16; per-pixel
                    # 882B contiguous runs, HWDGE on the scalar queue
                    wr = AP(
                        od_t.ap().tensor,
                        (2 * Y + sy) * (W * D) + sx * D,
                        [[2 * W * D, 4], [2 * D, 32], [1, D]],
                    )
                    nc.scalar.dma_start(out=wr, in_=osb[:])

    nc.compile()
    return nc


def _get_program():
    if "nc" not in _CACHE:
        _CACHE["nc"] = _build()
    return _CACHE["nc"]


def kernel(input1: np.ndarray, input2: np.ndarray) -> np.ndarray:
    from concourse import bass_utils

    nc = _get_program()
    input1 = np.ascontiguousarray(input1, dtype=np.float32)
    input2 = np.ascontiguousarray(input2, dtype=np.float32)
    B = input1.shape[0]
    in_maps = [{"i1": input1[b], "i2": input2[b]} for b in range(B)]
    res = bass_utils.run_bass_kernel_spmd(nc, in_maps, core_ids=list(range(B)))
    # [B, H, W, D] bf16 -> fp32 (exact widening), then to [B, D, H, W]
    out = np.stack([np.asarray(r["od"]).astype(np.float32) for r in res.results])
    return np.ascontiguousarray(out.transpose(0, 3, 1, 2))  # [B, D, H, W]


# revision 8
# speedup vs baseline: 2.5255x; 1.6372x over previous
"""FlowNet correlation kernel for Trainium2 (8 NeuronCores, batch-parallel).

Problem: out[b, d, y, x] = (1/C) * sum_c i1[b,c,y,x] * pad(i2)[b,c,y+dy,x+dx]
  B=8, C=256, H=48, W=64, pad=20, displacements dy,dx in {-20..20 step 2}
  (21x21 = 441), output [8, 441, 48, 64] fp32.

Strategy (per core, one batch element):
  Displacement stride 2 => 4 polyphase subproblems (y-parity sy, x-parity
  sx), each a dense +-10 correlation on a 24x32 quarter image. Inputs are
  cast to bf16 during the load DMA (tolerance is 2e-2 rms; bf16 lands
  ~3e-3). For each subproblem and 4-sub-row block (M = 128 pixels), bf16
  matmuls compute the all-pairs band restricted to live (in-image) window
  positions; structural zeros come from whole-buffer memsets of the
  geometry-keyed band buffers (dead regions are never overwritten).

  Extraction to [y, x, d] HBM layout is 3-stage. A direct diagonal gather
  DMA costs ~600ns fixed per DMA instruction plus ~2-3ns per 84B
  descriptor on the issuing engine's queue (measured), so both the
  descriptor count AND the DMA instruction count must be small:
    A. SBUF->SBUF DMA combining the px-diagonal shift (flat addressing:
       partition stride = row+1) with the py*52 window-row offset folded
       into the per-py-group source offset:
         sel1[p, j] = band[p, px + py*52 + j].
       Both sx parities and two y-geometries share one buffer, so one
       DMA per 32-partition py-group serves 4 blocks: 24 DMA instructions
       total with ~2.1KB contiguous runs.
    B. ONE partition-uniform compute copy per block:
         out_sb[p, sx, oy*21+ox] = sel1[p, sx, oy*52+ox]  (bf16).
    C. one linear DMA per sx-paired block: out_sb [128, 882] bf16 -> HBM
       bf16 with 1764B x-contiguous runs. The band is already bf16, so a
       bf16 HBM output loses nothing; the host widens to fp32 (exact) and
       transposes [H, W, D] -> [D, H, W].
"""

import numpy as np

C = 256
H, W = 48, 64
ND = 21          # displacements per axis
D = ND * ND      # 441
SUB_H, SUB_W = H // 2, W // 2      # 24, 32
BAND_W = 52                        # window cols per band row
BAND_N = 24 * BAND_W               # 1248 elems per (block, sx) band row
SEL_W = 20 * BAND_W + ND           # 1061: stage-B reads j in [0, SEL_W)
PAIR_BAND = 4 * BAND_N             # band_pair row: [g&1, sx, 1248]
PAIR_SEL = 4 * SEL_W               # sel1_pair row: [g&1, sx, 1061]
YS = [0, 4, 8, 12, 16, 20]
# live window-row range [wr0, wr1) per y-block (rows with in-image data)
LIVE = [(max(0, 10 - Y), min(24, 34 - Y)) for Y in YS]

_CACHE = {}


def _build():
    import concourse.bacc as bacc
    import concourse.mybir as mybir
    from concourse.bass_types import AP, SBTensorHandle
    from concourse.tile import TileContext
    from bass_rust import add_dep_helper

    f32 = mybir.dt.float32
    bf16 = mybir.dt.bfloat16

    def alias_sbuf(nc, name, shape, dtype, offset, base_partition):
        # SBUF tensor view at a fixed byte offset and nonzero base partition.
        # Rebases the partition origin so diagonal gather APs keep their flat
        # offset inside one partition row (walrus rejects partition-crossing
        # offsets on irregular APs).
        uname = nc._get_name(name, add_next_id=True)
        nc._tensor(uname, list(shape), dtype, type="SB")
        import functools, operator
        per_part = functools.reduce(operator.mul, shape[1:]) * mybir.dt.size(dtype)
        h = SBTensorHandle(
            uname,
            list(shape),
            dtype,
            base_partition=base_partition,
            manual_sbuf_range=(offset, offset + per_part),
            manual_base_name=name,
        )
        mloc = nc.lookup_mloc(h)
        mloc.allocated = True
        mloc.addr = offset
        mloc.base = base_partition
        return h

    nc = bacc.Bacc("TRN2", target_bir_lowering=False, debug=False)
    i1_t = nc.dram_tensor("i1", [C, H, W], f32, kind="ExternalInput")
    i2_t = nc.dram_tensor("i2", [C, H, W], f32, kind="ExternalInput")
    od_t = nc.dram_tensor("od", [H, W, D], bf16, kind="ExternalOutput")

    # band_pair[gp] holds bands for geometries {2gp, 2gp+1} x both sx;
    # sel1_pair[gp] the matching shifted copies. Raw tensors; deps manual.
    band = []
    band_alias = []
    sel1 = []
    for gp in range(3):
        h = nc.alloc_sbuf_tensor(f"band{gp}", [128, PAIR_BAND], bf16)
        addr = nc.lookup_mloc(h).addr
        band.append(h)
        band_alias.append(
            [
                alias_sbuf(
                    nc, f"band{gp}ry{ry}", [32, PAIR_BAND], bf16, addr, 32 * ry
                )
                for ry in range(4)
            ]
        )
        sel1.append(nc.alloc_sbuf_tensor(f"sel1_{gp}", [128, PAIR_SEL], bf16))

    # last readers of band[gp] (stage-A DMAs) / sel1[gp] (stage-B copies);
    # seeded with the zeroing memsets so first writers wait for them.
    band_last = [[] for _ in range(3)]
    sel1_last = [[] for _ in range(3)]

    inv_c = 1.0 / C

    with TileContext(nc) as tc:
        with (
            tc.tile_pool(name="inp", bufs=1) as inp_pool,
            tc.tile_pool(name="out", bufs=3) as out_pool,
            tc.tile_pool(name="ps", bufs=4, space="PSUM") as ps_pool,
        ):
            i1b = [
                inp_pool.tile([128, H * W], bf16, name=f"i1b{k}", tag=f"i1b{k}")
                for k in range(2)
            ]
            i2b = [
                inp_pool.tile([128, H * W], bf16, name=f"i2b{k}", tag=f"i2b{k}")
                for k in range(2)
            ]
            i1s = [
                [
                    inp_pool.tile(
                        [128, SUB_H * SUB_W], bf16, name=f"i1s{k}{s}", tag=f"i1s{k}{s}"
                    )
                    for s in range(4)
                ]
                for k in range(2)
            ]
            i1v = [t[:].rearrange("c (h w) -> c h w", h=H) for t in i1b]
            i2v = [t[:].rearrange("c (h w) -> c h w", h=H) for t in i2b]

            # band zeroing first (no deps; overlaps the input DMAs)
            for gp in range(3):
                eng = nc.vector if (gp & 1) else nc.gpsimd
                mb = eng.memset(band[gp].ap(), 0.0)
                band_last[gp] = [mb]

            # input loads: fp32 HBM -> bf16 SBUF, cast by SWDGE (gpsimd)
            for k in range(2):
                cs = slice(128 * k, 128 * (k + 1))
                nc.gpsimd.dma_start(out=i1b[k][:], in_=i1_t.ap()[cs])
                nc.gpsimd.dma_start(out=i2b[k][:], in_=i2_t.ap()[cs])
                # de-interleave i1 into the 4 polyphase sub-images
                # (stationary matmul operands need a single-stride free dim)
                for s in range(4):
                    sy, sx = s >> 1, s & 1
                    dst = i1s[k][s][:].rearrange("c (py px) -> c py px", py=SUB_H)
                    src = i1v[k][:, sy : sy + 2 * SUB_H - 1 : 2, sx::2]
                    if s & 1:
                        nc.scalar.copy(dst, src)
                    else:
                        nc.vector.tensor_copy(dst, src)

            for sy in range(2):
                for gp in range(3):
                    pair_data = []
                    for gi in range(2):
                        g = 2 * gp + gi
                        Y = YS[g]
                        wr0, wr1 = LIVE[g]
                        nr = wr1 - wr0
                        chunks = [(a, min(a + 16, nr)) for a in range(0, nr, 16)]
                        copies = []
                        for sx in range(2):
                            s = 2 * sy + sx
                            ps = ps_pool.tile([128, 1024], f32, name="ps")
                            for k in range(2):
                                lhs = i1s[k][s][:, 32 * Y : 32 * Y + 128]
                                for j, (a, b) in enumerate(chunks):
                                    r = Y + wr0 + a - 10  # 1st interior sub-row
                                    rhs = i2v[k][
                                        :,
                                        2 * r + sy : 2 * (r + b - a - 1) + sy + 1 : 2,
                                        sx::2,
                                    ]
                                    nc.tensor.matmul(
                                        ps[:, 512 * j : 512 * j + (b - a) * 32],
                                        lhsT=lhs,
                                        rhs=rhs,
                                        start=(k == 0),
                                        stop=(k == 1),
                                    )
                            # psum -> band slot (scale 1/C, cast to bf16)
                            eng = nc.vector if (sx == 0) else nc.scalar
                            dst = AP(
                                band[gp],
                                (2 * gi + sx) * BAND_N + wr0 * BAND_W + 10,
                                [[PAIR_BAND, 128], [BAND_W, nr], [1, 32]],
                            )
                            src = ps[:, 0 : nr * 32].rearrange(
                                "c (r w) -> c r w", r=nr
                            )
                            if eng is nc.vector:
                                cp = eng.tensor_scalar_mul(dst, src, inv_c)
                            else:
                                cp = eng.mul(dst, src, inv_c)
                            for rd in band_last[gp]:
                                add_dep_helper(cp.ins, rd.ins, reason="band WAR")
                            copies.append(cp)
                        pair_data.append((g, Y, copies))
                    # stage A: one diagonal-shift DMA per py-group serves all
                    # 4 (gi, sx) band slots: sel1[p, gi, sx, j] =
                    # band[p, gi, sx, px + py*52 + j], j in [0, SEL_W)
                    gathers = []
                    all_copies = [c for (_, _, cps) in pair_data for c in cps]
                    for ry in range(4):
                        rd = AP(
                            band_alias[gp][ry],
                            ry * BAND_W,
                            [[PAIR_BAND + 1, 32], [BAND_N, 4], [1, SEL_W]],
                        )
                        wr = AP(
                            sel1[gp],
                            (32 * ry) * PAIR_SEL,
                            [[PAIR_SEL, 32], [SEL_W, 4], [1, SEL_W]],
                        )
                        dma = nc.sync.dma_start(out=wr, in_=rd)
                        for cp in all_copies:
                            add_dep_helper(dma.ins, cp.ins, reason="band RAW")
                        for rb in sel1_last[gp]:
                            add_dep_helper(dma.ins, rb.ins, reason="sel1 WAR")
                        gathers.append(dma)
                    band_last[gp] = gathers
                    # stage B + C per gi-block (sx pair fused)
                    readers = []
                    for gi, (g, Y, _cps) in enumerate(pair_data):
                        osb = out_pool.tile([128, 2 * D], bf16, name="osb")
                        src = AP(
                            sel1[gp],
                            (2 * gi) * SEL_W,
                            [[PAIR_SEL, 128], [SEL_W, 2], [BAND_W, ND], [1, ND]],
                        )
                        dst = osb[:].rearrange("p (s a b) -> p s a b", s=2, a=ND)
                        eng = nc.vector if (gi == 0) else nc.scalar
                        if eng is nc.vector:
                            cp = eng.tensor_copy(dst, src)
                        else:
                            cp = eng.copy(dst, src)
                        for dma in gathers:
                            add_dep_helper(cp.ins, dma.ins, reason="sel1 RAW")
                        readers.append(cp)
                        # stage C: [y, x, d] bf16 store, 1764B runs (x pairs
                        # adjacent), on the otherwise-idle gpsimd queue
                        wr = AP(
                            od_t.ap().tensor,
                            (2 * Y + sy) * (W * D),
                            [[2 * W * D, 4], [2 * D, 32], [1, 2 * D]],
                        )
                        nc.gpsimd.dma_start(out=wr, in_=osb[:])
                    sel1_last[gp] = readers

    nc.compile()
    return nc


def _get_program():
    if "nc" not in _CACHE:
        _CACHE["nc"] = _build()
    return _CACHE["nc"]


def kernel(input1: np.ndarray, input2: np.ndarray) -> np.ndarray:
    from concourse import bass_utils

    nc = _get_program()
    input1 = np.ascontiguousarray(input1, dtype=np.float32)
    input2 = np.ascontiguousarray(input2, dtype=np.float32)
    B = input1.shape[0]
    in_maps = [{"i1": input1[b], "i2": input2[b]} for b in range(B)]
    res = bass_utils.run_bass_kernel_spmd(nc, in_maps, core_ids=list(range(B)))
    # [B, H, W, D] bf16 -> fp32 (exact widening), then to [B, D, H, W]
    out = np.stack([np.asarray(r["od"]).astype(np.float32) for r in res.results])
    return np.ascontiguousarray(out.transpose(0, 3, 1, 2))  # [B, D, H, W]
